# revision 9
# baseline (speedup 1.0000x reference)
"""Trainium2 Bass kernel for margin-ranking + weighted-BCE loss pair.

Math
----
reference margin loss (labels are 0/1):
  S_full := sum_{i,j in [B]^2} relu(m - prod_ij),  prod_ij = (p_i-p_j)(l_i-l_j)
  margin_loss = S_full/(2B) - relu(m)/2
prod is symmetric and zero for same-label pairs, so with d = p_pos - p_neg:
  S_full = m*(Npos^2 + Nneg^2) + 2 * sum_{i in pos, j in neg} relu(m - d_ij)

Device computation: with posm := pos - m (bf16-rounded on the host),
  relu(m - d) = relu(neg - posm) = max(neg, posm) - posm
so a 128-pos x 1024-neg tile of the cross grid is one fused max() op, and
the -posm shift is removed exactly on the host via the device-reduced posm
sum.  max(bf16, bf16) is exact, so the only rounding is f32 summation.

The 17 tiles per core are split across every engine that can touch data
(HW-measured rates; the DVE accumulate path runs at 1x, so production at
~0.4 ns/col is separated from reduction where possible):
  - 3 tiles: fused DVE tensor_scalar + accum (1 instr, 1.27us)
  - 9 tiles -> PE: ones-vector matmuls accumulate column sums into a
    [1,128] PSUM row (~0.82 ns/col pipelined with LDWEIGHTS)
  - 5 tiles -> Act: Copy-activation with accum_out (~1.41us)
  producers: DVE plain tensor_scalar (~0.48us) for 8 tiles, Pool
  tensor_scalar (~1.2us) for 6 tiles.

Pos is padded to 4352 with +16 sentinels, neg to 4096 with -16: sentinel
pairs have max(neg, posm) = posm exactly, i.e. contribute zero after the
correction.

BCE: bce_i = (1-t)z + (1+(pw-1)t)*softplus(-z), softplus(-z) = ln(1+exp(-z))
(safe: |z| tiny); device computes shard sums of softplus, t*softplus, t*z, z.

Distribution: core c = (q, h) owns pos half q (17 blocks of 128) x neg
quarter h (1024 cols).  Host does only permutation/padding/replication and
the final combine of per-core partial sums.
"""

import numpy as np
import ml_dtypes

import concourse.bacc as bacc
import concourse.bass as bass
import concourse.mybir as mybir
import concourse.tile as tile
from concourse.bass_utils import run_bass_kernel_spmd

B = 8192
NCORES = 8
PP = 4352                  # padded pos count: 34 blocks of 128, 17 per core
NN = 4096                  # padded neg count: 4 quarters of 1024
NB = 17                    # pos blocks per core
W = 1024                   # neg cols per core
SP = 16.0                  # pos sentinel
SN = -16.0                 # neg sentinel
BCE_N = B // NCORES        # 1024 -> [128, 8]
BCE_F = BCE_N // 128

# (producer, consumer) role per block, in emission order.  producers:
# d=DVE tensor_scalar, p=Pool tensor_scalar.  consumers: P=PE ones-matmul,
# A=Act copy+accum, F=fused DVE tensor_scalar+accum (producer ignored).
ROLES = [("p", "P"), ("d", "P"), ("p", "A"), ("d", "F"), ("d", "P"),
         ("p", "P"), ("d", "A"), ("d", "P"), ("p", "A"), ("d", "F"),
         ("d", "P"), ("p", "P"), ("d", "A"), ("d", "P"), ("p", "P"),
         ("d", "F"), ("d", "P")]
assert len(ROLES) == NB
N_PE = sum(1 for _, c in ROLES if c == "P")
N_ACT = sum(1 for _, c in ROLES if c == "A")
N_F = sum(1 for _, c in ROLES if c == "F")
NMARG = N_ACT + N_F        # margin accum cols (PE blocks go to the psum row)
# acc col layout: [0:NMARG) margin, then sp, tsp, tz, z
NACC = NMARG + 4

f32 = mybir.dt.float32
bf16 = mybir.dt.bfloat16


def _build_program(margin: float):
    from contextlib import ExitStack

    assert 0.0 <= margin <= 8.0, "sentinel padding assumes 0 <= margin <= 8"
    nc = bacc.Bacc("TRN2", target_bir_lowering=False, debug=False,
                   num_devices=NCORES)
    Copy = mybir.ActivationFunctionType.Copy
    Exp = mybir.ActivationFunctionType.Exp
    Ln = mybir.ActivationFunctionType.Ln
    add = mybir.AluOpType.add
    mult = mybir.AluOpType.mult
    amax = mybir.AluOpType.max
    bypass = mybir.AluOpType.bypass

    negrep_d = nc.dram_tensor("negrep", [128, W], bf16, kind="ExternalInput")
    posm_d = nc.dram_tensor("posm", [128, NB], f32, kind="ExternalInput")
    blg_d = nc.dram_tensor("blg", [128, BCE_F], f32, kind="ExternalInput")
    btg_d = nc.dram_tensor("btg", [128, BCE_F], f32, kind="ExternalInput")
    out_d = nc.dram_tensor("out", [1, 7], f32, kind="ExternalOutput")

    with tile.TileContext(nc) as tc, ExitStack() as ctx:
        small = ctx.enter_context(tc.tile_pool(name="small", bufs=1))
        dpool = ctx.enter_context(tc.tile_pool(name="dpool", bufs=3))
        ppool = ctx.enter_context(tc.tile_pool(name="ppool", bufs=3))
        spool = ctx.enter_context(tc.tile_pool(name="spool", bufs=2))
        psmall = ctx.enter_context(
            tc.tile_pool(name="psmall", bufs=1, space=bass.MemorySpace.PSUM))

        # ---- input DMAs on separate queues -------------------------------
        negrep = small.tile([128, W], bf16, tag="negrep")
        posm = small.tile([128, NB], f32, tag="posm")
        zt = small.tile([128, BCE_F], f32, tag="zt")
        tt = small.tile([128, BCE_F], f32, tag="tt")
        nc.sync.dma_start(out=negrep[:, :], in_=negrep_d[:, :])
        nc.scalar.dma_start(out=posm[:, :], in_=posm_d[:, :])
        nc.scalar.dma_start(out=zt[:, :], in_=blg_d[:, :])
        nc.scalar.dma_start(out=tt[:, :], in_=btg_d[:, :])

        onesf = small.tile([128, 1], f32, tag="onesf")
        onesb = small.tile([128, 1], bf16, tag="onesb")
        nc.gpsimd.memset(onesf[:, :], 1.0)
        nc.gpsimd.memset(onesb[:, :], 1.0)

        acc = small.tile([128, NACC], f32, tag="acc")

        # ---- BCE on the 1024-element shard -------------------------------
        # softplus(-z) = ln(1 + exp(-z)); |z| small so exp(-z) cannot
        # overflow f32 for this problem's logits
        sp = small.tile([128, BCE_F], f32, tag="sp")
        e1 = small.tile([128, BCE_F], f32, tag="e1")
        scr8a = small.tile([128, BCE_F], f32, tag="scr8a")
        scr8b = small.tile([128, BCE_F], f32, tag="scr8b")
        nc.scalar.activation(e1[:, :], zt[:, :], Exp, scale=-1.0)
        nc.scalar.activation(sp[:, :], e1[:, :], Ln, bias=1.0,
                             accum_out=acc[:, NMARG + 0: NMARG + 1])
        nc.vector.scalar_tensor_tensor(
            scr8a[:, :], tt[:, :], 1.0, sp[:, :], bypass, mult,
            accum_out=acc[:, NMARG + 1: NMARG + 2])
        nc.vector.scalar_tensor_tensor(
            scr8b[:, :], tt[:, :], 1.0, zt[:, :], bypass, mult,
            accum_out=acc[:, NMARG + 2: NMARG + 3])
        nc.vector.tensor_reduce(acc[:, NMARG + 3: NMARG + 4], zt[:, :],
                                axis=mybir.AxisListType.X, op=add)

        # ---- the 17 pos-block x 1024-neg-col grid tiles ------------------
        # tile[p, n] = max(negrep[n], posm[p, b]); sum(tile) - W*posm is the
        # block's relu sum (host removes the shift via the posm reduction)
        pesum = psmall.tile([1, 128], f32, tag="pesum")
        n_pe_mm = N_PE * (W // 128)
        im = 0
        imarg = 0
        for b, (prod, cons) in enumerate(ROLES):
            pv = posm[:, b: b + 1]
            if cons == "F":
                scr = spool.tile([128, W], bf16, tag="scr")
                nc.vector.tensor_scalar(scr[:, :], negrep[:, :], pv,
                                        0.0, amax, add,
                                        accum_out=acc[:, imarg: imarg + 1])
                imarg += 1
                continue
            if prod == "p":
                dt = ppool.tile([128, W], bf16, tag="ptile")
                nc.gpsimd.tensor_scalar(dt[:, :], negrep[:, :], pv,
                                        0.0, amax, add)
            else:
                dt = dpool.tile([128, W], bf16, tag="dtile")
                nc.vector.tensor_scalar(dt[:, :], negrep[:, :], pv,
                                        0.0, amax, add)
            if cons == "P":
                for j in range(W // 128):
                    nc.tensor.matmul(pesum[:, :], onesb[:, :],
                                     dt[:, 128 * j: 128 * (j + 1)],
                                     start=(im == 0), stop=(im == n_pe_mm - 1))
                    im += 1
            else:  # Act copy+accum
                ascr = spool.tile([128, W], bf16, tag="ascr")
                nc.scalar.activation(ascr[:, :], dt[:, :], Copy,
                                     accum_out=acc[:, imarg: imarg + 1])
                imarg += 1
        assert imarg == NMARG and im == n_pe_mm

        # ---- final reduction --------------------------------------------
        stacked = small.tile([128, 6], f32, tag="stacked")
        nc.vector.tensor_reduce(stacked[:, 0:1], acc[:, 0:NMARG],
                                axis=mybir.AxisListType.X, op=add)
        nc.vector.tensor_copy(stacked[:, 1:5], acc[:, NMARG:NMARG + 4])
        nc.vector.tensor_reduce(stacked[:, 5:6], posm[:, :],
                                axis=mybir.AxisListType.X, op=add)
        pfin = psmall.tile([1, 6], f32, tag="pfin")
        nc.tensor.matmul(pfin[:, :], onesf[:, :], stacked[:, :],
                         start=True, stop=True)
        outt = small.tile([1, 7], f32, tag="outt")
        nc.scalar.copy(outt[:, 0:6], pfin[:, :])
        red1 = small.tile([1, 1], f32, tag="red1")
        nc.vector.tensor_reduce(red1[:, :], pesum[:, :],
                                axis=mybir.AxisListType.X, op=add)
        nc.vector.tensor_copy(outt[:, 6:7], red1[:, :])
        nc.sync.dma_start(out=out_d[:, :], in_=outt[:, :])

    nc.compile()
    return nc


_programs: dict = {}


def _get_program(margin: float):
    if margin not in _programs:
        _programs[margin] = _build_program(margin)
    return _programs[margin]


def _make_in_maps(preds, labels, logits, targets, margin):
    p = np.ascontiguousarray(np.asarray(preds, np.float32))
    l = np.ascontiguousarray(np.asarray(labels, np.float32))
    z = np.ascontiguousarray(np.asarray(logits, np.float32))
    tg = np.ascontiguousarray(np.asarray(targets, np.float32))
    ndt = ml_dtypes.bfloat16

    mask = l >= 0.5
    npos = int(mask.sum())
    nneg = B - npos
    assert npos <= PP and nneg <= NN, (npos, nneg)
    pos_pad = np.full(PP, SP, np.float32)
    pos_pad[:npos] = p[mask]
    neg_pad = np.full(NN, SN, np.float32)
    neg_pad[:nneg] = p[~mask]
    neg16 = neg_pad.astype(ndt)
    # posm rounded to bf16 so that max(neg16, posm) on device is exact
    posm = (pos_pad - np.float32(margin)).astype(ndt).astype(np.float32)

    in_maps = []
    for c in range(NCORES):
        q, h = divmod(c, 4)
        pv = posm[q * NB * 128:(q + 1) * NB * 128].reshape(NB, 128).T
        negs = neg16[h * W:(h + 1) * W]
        in_maps.append({
            "negrep": np.ascontiguousarray(np.broadcast_to(negs, (128, W))),
            "posm": np.ascontiguousarray(pv),
            "blg": z[BCE_N * c: BCE_N * (c + 1)].reshape(128, BCE_F).copy(),
            "btg": tg[BCE_N * c: BCE_N * (c + 1)].reshape(128, BCE_F).copy(),
        })
    return in_maps, npos, nneg


def _combine(outs, npos, nneg, margin, pw):
    # outs: [NCORES, 1, 7]: cols 0 sum(max) from acc cols, 1 sp, 2 t*sp,
    # 3 t*z, 4 z, 5 sum(posm), 6 sum(max) from the PE psum row
    o = outs.astype(np.float64)
    m = float(margin)
    s_grid = (o[:, 0, 0] + o[:, 0, 6] - W * o[:, 0, 5]).sum()
    s_full = m * (npos * npos + nneg * nneg) + 2.0 * s_grid
    margin_loss = s_full / (2.0 * B) - max(m, 0.0) / 2.0
    s_bce = (o[:, 0, 4].sum() - o[:, 0, 3].sum() + o[:, 0, 1].sum()
             + (pw - 1.0) * o[:, 0, 2].sum())
    return np.array([margin_loss, s_bce / B], dtype=np.float32)


def _run(inputs: dict, trace: bool = False, **spmd_kwargs):
    m = float(np.asarray(inputs["margin"]))
    pw = float(np.asarray(inputs["pos_weight"], np.float32).reshape(-1)[0])
    nc = _get_program(m)
    in_maps, npos, nneg = _make_in_maps(inputs["preds"], inputs["labels"],
                                        inputs["logits"], inputs["targets"],
                                        m)
    res = run_bass_kernel_spmd(nc, in_maps, core_ids=list(range(NCORES)),
                               trace=trace, **spmd_kwargs)
    outs = np.stack([np.asarray(r["out"], np.float32) for r in res.results])
    return _combine(outs, npos, nneg, m, pw), res


def kernel(preds, labels, logits, targets, pos_weight, margin):
    out, _ = _run(dict(preds=preds, labels=labels, logits=logits,
                       targets=targets, pos_weight=pos_weight,
                       margin=margin))
    return out


# revision 17
# speedup vs baseline: 4.4681x; 4.4681x over previous
"""Trainium2 Bass kernel for margin-ranking + weighted-BCE loss pair.

Math
----
reference margin loss (labels are 0/1):
  S_full := sum_{i,j in [B]^2} relu(m - prod_ij),  prod_ij = (p_i-p_j)(l_i-l_j)
  margin_loss = S_full/(2B) - relu(m)/2
prod is symmetric and zero for same-label pairs, so with d = p_pos - p_neg:
  S_full = m*(Npos^2 + Nneg^2) + 2 * sum_{i in pos, j in neg} relu(m - d_ij)

Device computation: with posm := pos - m (bf16-rounded on the host),
  relu(m - d) = relu(neg - posm) = max(neg, posm) - posm
so a 128-pos x 1024-neg tile of the cross grid is one fused max() op, and
the -posm shift is removed exactly on the host via the device-reduced posm
sum.  max(bf16, bf16) is exact, so the only rounding is f32 summation.

The 17 tiles per core are split across the three engines that can do this
work at speed (HW-measured rates; the DVE accumulate path runs at 1x, so
production at ~0.47us/tile is separated from reduction where possible):
  - 10 tiles: DVE plain tensor_scalar produces the max-tile (~0.47us, 4x
    perf mode), PE ones-vector matmuls accumulate column sums into a
    [1,128] PSUM row (~0.84us/tile, LDWEIGHTS pipelined with MATMUL)
  - 5 tiles: fused Act activation relu(negrep - posm) with bias=-posm and
    accum_out (~1.41us; exact relu, no posm correction)
  - 2 tiles: fused DVE tensor_scalar + accum (1 instr, ~1.27us)
(Pool's AP-scalar tensor_scalar measured ~15us/tile - unusable.)

Pos is padded to 4352 with +16 sentinels, neg to 4096 with -16: sentinel
pairs have max(neg, posm) = posm exactly, i.e. contribute zero after the
correction.

BCE: bce_i = (1-t)z + (1+(pw-1)t)*softplus(-z), softplus(-z) = ln(1+exp(-z))
(safe: |z| tiny); device computes shard sums of softplus, t*softplus, t*z, z.

Distribution: core c = (q, h) owns pos half q (17 blocks of 128) x neg
quarter h (1024 cols).  Host does only permutation/padding/replication and
the final combine of per-core partial sums.
"""

import numpy as np
import ml_dtypes

import concourse.bacc as bacc
import concourse.bass as bass
import concourse.mybir as mybir
import concourse.tile as tile
from concourse.bass_utils import run_bass_kernel_spmd

B = 8192
NCORES = 8
PP = 4352                  # padded pos count: 34 blocks of 128, 17 per core
NN = 4096                  # padded neg count: 4 quarters of 1024
NB = 17                    # pos blocks per core
W = 1024                   # neg cols per core
SP = 16.0                  # pos sentinel
SN = -16.0                 # neg sentinel
BCE_N = B // NCORES        # 1024 -> [128, 8]
BCE_F = BCE_N // 128

# (role, posm col) per block in emission order.  P: DVE tensor_scalar
# produces the max-tile, PE ones-matmuls sum it into the psum row.
# F: fused DVE tensor_scalar with accum_out.  A: fused Act activation
# relu(negrep - posm) with accum_out (exact relu; no posm correction).
# P/F blocks use posm cols [0:12) so the correction reduce is contiguous.
ROLES = [("A", 12), ("P", 0), ("P", 1), ("A", 13), ("P", 2), ("F", 10),
         ("P", 3), ("P", 4), ("A", 14), ("P", 5), ("F", 11), ("P", 6),
         ("A", 15), ("P", 7), ("P", 8), ("A", 16), ("P", 9)]
assert len(ROLES) == NB
assert sorted(b for _, b in ROLES) == list(range(NB))
N_PE = sum(1 for r, _ in ROLES if r == "P")
N_ACT = sum(1 for r, _ in ROLES if r == "A")
N_F = sum(1 for r, _ in ROLES if r == "F")
NCORR = N_PE + N_F         # blocks needing the W*posm correction (cols 0..)
NMARG = N_ACT + N_F        # margin accum cols (PE blocks go to the psum row)
# acc col layout: [0:NMARG) margin, then sp, tsp, tz, z
NACC = NMARG + 4

f32 = mybir.dt.float32
bf16 = mybir.dt.bfloat16


def _build_program(margin: float):
    from contextlib import ExitStack

    assert 0.0 <= margin <= 8.0, "sentinel padding assumes 0 <= margin <= 8"
    nc = bacc.Bacc("TRN2", target_bir_lowering=False, debug=False,
                   num_devices=NCORES)
    Relu = mybir.ActivationFunctionType.Relu
    Exp = mybir.ActivationFunctionType.Exp
    Ln = mybir.ActivationFunctionType.Ln
    add = mybir.AluOpType.add
    mult = mybir.AluOpType.mult
    amax = mybir.AluOpType.max
    bypass = mybir.AluOpType.bypass

    negrep_d = nc.dram_tensor("negrep", [128, W], bf16, kind="ExternalInput")
    posm_d = nc.dram_tensor("posm", [128, NB], f32, kind="ExternalInput")
    blg_d = nc.dram_tensor("blg", [128, BCE_F], f32, kind="ExternalInput")
    btg_d = nc.dram_tensor("btg", [128, BCE_F], f32, kind="ExternalInput")
    out_d = nc.dram_tensor("out", [1, 7], f32, kind="ExternalOutput")

    with tile.TileContext(nc) as tc, ExitStack() as ctx:
        small = ctx.enter_context(tc.tile_pool(name="small", bufs=1))
        dpool = ctx.enter_context(tc.tile_pool(name="dpool", bufs=3))
        spool = ctx.enter_context(tc.tile_pool(name="spool", bufs=2))
        psmall = ctx.enter_context(
            tc.tile_pool(name="psmall", bufs=1, space=bass.MemorySpace.PSUM))

        # ---- input DMAs on separate queues -------------------------------
        negrep = small.tile([128, W], bf16, tag="negrep")
        posm = small.tile([128, NB], f32, tag="posm")
        zt = small.tile([128, BCE_F], f32, tag="zt")
        tt = small.tile([128, BCE_F], f32, tag="tt")
        nc.sync.dma_start(out=negrep[:, :], in_=negrep_d[:, :])
        nc.scalar.dma_start(out=posm[:, :], in_=posm_d[:, :])
        nc.scalar.dma_start(out=zt[:, :], in_=blg_d[:, :])
        nc.scalar.dma_start(out=tt[:, :], in_=btg_d[:, :])

        onesf = small.tile([128, 1], f32, tag="onesf")
        onesb = small.tile([128, 1], bf16, tag="onesb")
        nc.gpsimd.memset(onesf[:, :], 1.0)
        nc.gpsimd.memset(onesb[:, :], 1.0)
        # negposm = -posm, the per-partition relu bias for the Act blocks
        negposm = small.tile([128, NB], f32, tag="negposm")
        nc.vector.tensor_scalar_mul(negposm[:, :], posm[:, :], -1.0)

        acc = small.tile([128, NACC], f32, tag="acc")

        # ---- BCE on the 1024-element shard -------------------------------
        # softplus(-z) = ln(1 + exp(-z)); |z| small so exp(-z) cannot
        # overflow f32 for this problem's logits
        sp = small.tile([128, BCE_F], f32, tag="sp")
        e1 = small.tile([128, BCE_F], f32, tag="e1")
        scr8a = small.tile([128, BCE_F], f32, tag="scr8a")
        scr8b = small.tile([128, BCE_F], f32, tag="scr8b")
        nc.scalar.activation(e1[:, :], zt[:, :], Exp, scale=-1.0)
        nc.scalar.activation(sp[:, :], e1[:, :], Ln, bias=1.0,
                             accum_out=acc[:, NMARG + 0: NMARG + 1])
        nc.vector.scalar_tensor_tensor(
            scr8a[:, :], tt[:, :], 1.0, sp[:, :], bypass, mult,
            accum_out=acc[:, NMARG + 1: NMARG + 2])
        nc.vector.scalar_tensor_tensor(
            scr8b[:, :], tt[:, :], 1.0, zt[:, :], bypass, mult,
            accum_out=acc[:, NMARG + 2: NMARG + 3])
        nc.vector.tensor_reduce(acc[:, NMARG + 3: NMARG + 4], zt[:, :],
                                axis=mybir.AxisListType.X, op=add)

        # ---- the 17 pos-block x 1024-neg-col grid tiles ------------------
        # tile[p, n] = max(negrep[n], posm[p, b]); sum(tile) - W*posm is the
        # block's relu sum (host removes the shift via the posm reduction)
        pesum = psmall.tile([1, 128], f32, tag="pesum")
        n_pe_mm = N_PE * (W // 128)
        im = 0
        imarg = 0
        for role, b in ROLES:
            pv = posm[:, b: b + 1]
            if role == "F":
                scr = spool.tile([128, W], bf16, tag="scr")
                nc.vector.tensor_scalar(scr[:, :], negrep[:, :], pv,
                                        0.0, amax, add,
                                        accum_out=acc[:, imarg: imarg + 1])
                imarg += 1
            elif role == "A":
                ascr = spool.tile([128, W], bf16, tag="ascr")
                nc.scalar.activation(ascr[:, :], negrep[:, :], Relu,
                                     bias=negposm[:, b: b + 1],
                                     accum_out=acc[:, imarg: imarg + 1])
                imarg += 1
            else:  # P: DVE produces, PE ones-matmuls consume
                dt = dpool.tile([128, W], bf16, tag="dtile")
                nc.vector.tensor_scalar(dt[:, :], negrep[:, :], pv,
                                        0.0, amax, add)
                for j in range(W // 128):
                    nc.tensor.matmul(pesum[:, :], onesb[:, :],
                                     dt[:, 128 * j: 128 * (j + 1)],
                                     start=(im == 0), stop=(im == n_pe_mm - 1))
                    im += 1
        assert imarg == NMARG and im == n_pe_mm

        # ---- final reduction --------------------------------------------
        stacked = small.tile([128, 6], f32, tag="stacked")
        nc.vector.tensor_reduce(stacked[:, 0:1], acc[:, 0:NMARG],
                                axis=mybir.AxisListType.X, op=add)
        nc.vector.tensor_copy(stacked[:, 1:5], acc[:, NMARG:NMARG + 4])
        nc.vector.tensor_reduce(stacked[:, 5:6], posm[:, 0:NCORR],
                                axis=mybir.AxisListType.X, op=add)
        pfin = psmall.tile([1, 6], f32, tag="pfin")
        nc.tensor.matmul(pfin[:, :], onesf[:, :], stacked[:, :],
                         start=True, stop=True)
        outt = small.tile([1, 7], f32, tag="outt")
        nc.scalar.copy(outt[:, 0:6], pfin[:, :])
        red1 = small.tile([1, 1], f32, tag="red1")
        nc.vector.tensor_reduce(red1[:, :], pesum[:, :],
                                axis=mybir.AxisListType.X, op=add)
        nc.vector.tensor_copy(outt[:, 6:7], red1[:, :])
        nc.sync.dma_start(out=out_d[:, :], in_=outt[:, :])

    nc.compile()
    return nc


_programs: dict = {}


def _get_program(margin: float):
    if margin not in _programs:
        _programs[margin] = _build_program(margin)
    return _programs[margin]


def _make_in_maps(preds, labels, logits, targets, margin):
    p = np.ascontiguousarray(np.asarray(preds, np.float32))
    l = np.ascontiguousarray(np.asarray(labels, np.float32))
    z = np.ascontiguousarray(np.asarray(logits, np.float32))
    tg = np.ascontiguousarray(np.asarray(targets, np.float32))
    ndt = ml_dtypes.bfloat16

    mask = l >= 0.5
    npos = int(mask.sum())
    nneg = B - npos
    assert npos <= PP and nneg <= NN, (npos, nneg)
    pos_pad = np.full(PP, SP, np.float32)
    pos_pad[:npos] = p[mask]
    neg_pad = np.full(NN, SN, np.float32)
    neg_pad[:nneg] = p[~mask]
    neg16 = neg_pad.astype(ndt)
    # posm rounded to bf16 so that max(neg16, posm) on device is exact
    posm = (pos_pad - np.float32(margin)).astype(ndt).astype(np.float32)

    in_maps = []
    for c in range(NCORES):
        q, h = divmod(c, 4)
        pv = posm[q * NB * 128:(q + 1) * NB * 128].reshape(NB, 128).T
        negs = neg16[h * W:(h + 1) * W]
        in_maps.append({
            "negrep": np.ascontiguousarray(np.broadcast_to(negs, (128, W))),
            "posm": np.ascontiguousarray(pv),
            "blg": z[BCE_N * c: BCE_N * (c + 1)].reshape(128, BCE_F).copy(),
            "btg": tg[BCE_N * c: BCE_N * (c + 1)].reshape(128, BCE_F).copy(),
        })
    return in_maps, npos, nneg


def _combine(outs, npos, nneg, margin, pw):
    # outs: [NCORES, 1, 7]: cols 0 margin acc cols (A relu sums + F max
    # sums), 1 sp, 2 t*sp, 3 t*z, 4 z, 5 sum(posm over P/F cols),
    # 6 sum(max) from the PE psum row
    o = outs.astype(np.float64)
    m = float(margin)
    s_grid = (o[:, 0, 0] + o[:, 0, 6] - W * o[:, 0, 5]).sum()
    s_full = m * (npos * npos + nneg * nneg) + 2.0 * s_grid
    margin_loss = s_full / (2.0 * B) - max(m, 0.0) / 2.0
    s_bce = (o[:, 0, 4].sum() - o[:, 0, 3].sum() + o[:, 0, 1].sum()
             + (pw - 1.0) * o[:, 0, 2].sum())
    return np.array([margin_loss, s_bce / B], dtype=np.float32)


def _run(inputs: dict, trace: bool = False, **spmd_kwargs):
    m = float(np.asarray(inputs["margin"]))
    pw = float(np.asarray(inputs["pos_weight"], np.float32).reshape(-1)[0])
    nc = _get_program(m)
    in_maps, npos, nneg = _make_in_maps(inputs["preds"], inputs["labels"],
                                        inputs["logits"], inputs["targets"],
                                        m)
    res = run_bass_kernel_spmd(nc, in_maps, core_ids=list(range(NCORES)),
                               trace=trace, **spmd_kwargs)
    outs = np.stack([np.asarray(r["out"], np.float32) for r in res.results])
    return _combine(outs, npos, nneg, m, pw), res


def kernel(preds, labels, logits, targets, pos_weight, margin):
    out, _ = _run(dict(preds=preds, labels=labels, logits=logits,
                       targets=targets, pos_weight=pos_weight,
                       margin=margin))
    return out
